# revision 1
# baseline (speedup 1.0000x reference)
"""MixedScoreMultiHeadAttention TRN2 kernel, v2.

Data-parallel over batch: 32 batches -> 8 cores x 4 batches (BL=4).

Host-side prep (free): x transposed to [E, TOK] fp16, Wq/Wk padded to
32-col head slots and packed fp16, cost flattened r-major fp16, layer1
stationary [17, 256], layer2 block-diag sign/weight matrix, bias cols.

Device pipeline per core:
  proj q/k  -> quad tiles [128, TOK] fp16 (4 head-slots x 32 rows)
  dots      -> x4all [r, (h, b, c)] fp16 (PE, per head-pair psum)
  vhat      -> [c, (h, d|1)] fp16 with ones column for softmax denom
  per b: rhs assembly (2 DMAs) -> [17, 16384] fp16 (16 dot rows + cost)
  per (b, half): 16x { 2 layer1 mm [17,128]x[17,1024] -> psum [128,1024],
                       relu+bias evac (DVE/ACT balanced) -> rr fp16,
                       8 layer2 mm rr[:,128k]x w2l -> ps2[c,(r,h)] }
                 2x exp evac [128,512] -> wsb fp16
                 16 AV mm wsb[c,r-strided] x vhat -> psa [r, (h,17)]
                 reciprocal + per-head normalize -> fout
  out DMA per b.
"""
import sys

sys.path.insert(0, "/opt/trn_rl_repo")

import numpy as np
from contextlib import ExitStack

import concourse.bass as bass
import concourse.mybir as mybir
import concourse.tile as tile
from concourse import bacc
from concourse.bass_utils import run_bass_kernel_spmd

B, R, C, E, H, D, MS = 32, 128, 128, 256, 16, 16, 16
NCORES = 8
BL = B // NCORES   # 4 batches per core
TOK = BL * R       # 512 tokens per core per side
PTS = R * C        # 16384 points per batch

FP32 = mybir.dt.float32
FP16 = mybir.dt.float16
AF = mybir.ActivationFunctionType
ALU = mybir.AluOpType


class Balancer:
    """Greedy DVE/ACT load balancing for PSUM-exit elementwise ops."""

    def __init__(self, nc):
        self.nc = nc
        self.load = {"D": 0.0, "A": 0.0}

    @staticmethod
    def _cost(eng, n, psum_src):
        if eng == "D":
            return n * 1.0417 + (125.0 if psum_src else 60.0)
        return n * 0.8333 + 185.0

    def _pick(self, n, psum_src, forbid=()):
        e = min((x for x in "DA" if x not in forbid),
                key=lambda x: self.load[x])
        self.load[e] += self._cost(e, n, psum_src)
        return e

    def charge(self, eng, n, psum_src=True):
        self.load[eng] += self._cost(eng, n, psum_src)

    def copy(self, out, in_, n):
        self._ci = getattr(self, "_ci", 0) + 1
        if self._ci % 2 == 1:
            self.charge("D", n)
            self.nc.vector.tensor_copy(out, in_)
        else:
            self.charge("A", n)
            self.nc.scalar.copy(out, in_)

    def relu(self, out, in_, bias_ap, n, forbid=()):
        if self._pick(n, True, forbid) == "D":
            self.nc.vector.tensor_scalar(out, in_, bias_ap, 0.0,
                                         ALU.add, ALU.max)
        else:
            self.nc.scalar.activation(out, in_, AF.Relu, bias=bias_ap)


def build_kernel():
    nc = bacc.Bacc("TRN2", target_bir_lowering=False, debug=False,
                   num_devices=NCORES)

    xr_d = nc.dram_tensor("xrT", [128, 2 * TOK], FP16,
                          kind="ExternalInput").ap()
    xc_d = nc.dram_tensor("xcT", [128, 2 * TOK], FP16,
                          kind="ExternalInput").ap()
    wqk_d = nc.dram_tensor("wqk", [128, 2 * 2 * 768], FP16,
                           kind="ExternalInput").ap()
    wv_d = nc.dram_tensor("wv", [128, 2 * E], FP16,
                          kind="ExternalInput").ap()
    w1_d = nc.dram_tensor("w1l", [17, 256], FP16, kind="ExternalInput").ap()
    w2_d = nc.dram_tensor("w2l", [128, 16], FP16, kind="ExternalInput").ap()
    bc_d = nc.dram_tensor("bcol2", [128, 2], FP32, kind="ExternalInput").ap()
    cost_d = nc.dram_tensor("cost16", [BL, PTS], FP16,
                            kind="ExternalInput").ap()
    out_d = nc.dram_tensor("out", [BL, R, H * D], FP32,
                           kind="ExternalOutput").ap()
    # DRAM bounce for dot-score transposition, one tensor per head-quad
    # per batch-pair (b0/b1 critical-path, b2/b3 deferred)
    scr01s = [nc.dram_tensor(f"scr01_{q}", [128, 4 * 2 * C], FP16,
                             kind="Internal").ap() for q in range(4)]
    scr23s = [nc.dram_tensor(f"scr23_{q}", [128, 4 * 2 * C], FP16,
                             kind="Internal").ap() for q in range(4)]

    with tile.TileContext(nc) as tc, ExitStack() as ctx:
        const_p = ctx.enter_context(tc.tile_pool(name="const", bufs=1))
        big_p = ctx.enter_context(tc.tile_pool(name="big", bufs=1))
        rhs_p = ctx.enter_context(tc.tile_pool(name="rhs", bufs=2))
        rr_p = ctx.enter_context(tc.tile_pool(name="rr", bufs=8))
        wsb_p = ctx.enter_context(tc.tile_pool(name="wsb", bufs=4))
        fout_p = ctx.enter_context(tc.tile_pool(name="fout", bufs=1))
        small_p = ctx.enter_context(tc.tile_pool(name="small", bufs=4))
        psA = ctx.enter_context(
            tc.tile_pool(name="psA", bufs=3, space="PSUM"))   # [128,1024] x3
        ps2_p = ctx.enter_context(
            tc.tile_pool(name="ps2", bufs=2, space="PSUM"))   # [128,512] x2

        bal = Balancer(nc)

        # ---- PE warmup: ~4us of dummy matmuls on a zeroed tile so the
        # p-state ramp (2-4x slower first 3us) completes before real work
        zwarm = const_p.tile([128, 512], FP16)
        nc.gpsimd.memset(zwarm[:], 0.0)
        wps = ps2_p.tile([128, 512], FP32, tag="ps2", name="wps")
        for i in range(7):
            nc.tensor.matmul(wps[:], zwarm[:, 0:128], zwarm[:],
                             start=(i == 0), stop=(i == 6))

        # ---- const loads (single DMAs, host-prepped layouts)
        xr2 = const_p.tile([128, 2 * TOK], FP16)   # [p, (eh, tok)]
        xc2 = const_p.tile([128, 2 * TOK], FP16)
        wqk16 = const_p.tile([128, 2 * 2 * 768], FP16)  # [p, (eh, q768|k768)]
        wv16 = const_p.tile([128, 2 * E], FP16)         # [p, (eh, 256)]
        # load order: q-proj needs only the q-halves of wqk + xr
        nc.sync.dma_start(wqk16[:, 0:768], wqk_d[:, 0:768])
        nc.sync.dma_start(xr2[:], xr_d[:])
        nc.sync.dma_start(wqk16[:, 1536:2304], wqk_d[:, 1536:2304])
        nc.sync.dma_start(xc2[:], xc_d[:])
        nc.sync.dma_start(wqk16[:, 768:1536], wqk_d[:, 768:1536])
        nc.sync.dma_start(wqk16[:, 2304:3072], wqk_d[:, 2304:3072])
        w1sb = const_p.tile([17, 256], FP16)
        nc.gpsimd.dma_start(w1sb[:], w1_d[:])
        w2sb = const_p.tile([128, 16], FP16)
        nc.gpsimd.dma_start(w2sb[:], w2_d[:])
        bc2 = const_p.tile([128, 2], FP32)
        nc.gpsimd.dma_start(bc2[:], bc_d[:])
        nc.gpsimd.dma_start(wv16[:], wv_d[:])

        # ---- prep, mh-major: proj(q,k) for mh then dots for its 4 heads,
        #      so scratch DMAs start as early as possible
        quads = {}
        x4all = big_p.tile([128, H * BL * C], FP16, name="x4all")

        def headT(proj, h):
            q6, sl = h // 3, h % 3
            t = quads[(proj, q6 // 2)]
            co = (q6 % 2) * TOK
            return t[sl * 32:sl * 32 + 16, co:co + TOK]

        # rhs tiles for b0/b1 up front; cost rows load immediately, dot
        # rows arrive as quad-sized stage2 DMAs pipelined into the mh loop
        # views [r, h4, b2, c] for the stage2 per-batch slices
        s01v = [sd[:].rearrange("r (h b c) -> r h b c", h=4, b=2)
                for sd in scr01s]
        s23v = [sd[:].rearrange("r (h b c) -> r h b c", h=4, b=2)
                for sd in scr23s]
        rhs0 = rhs_p.tile([17, PTS], FP16, tag="rhs", name="rhs0")
        rhs1 = rhs_p.tile([17, PTS], FP16, tag="rhs", name="rhs1")
        nc.scalar.dma_start(rhs0[16:17, :], cost_d[0:1, :])
        nc.scalar.dma_start(rhs1[16:17, :], cost_d[1:2, :])

        def emit_proj(pt):
            # pairtile pt covers 6-quads 2pt, 2pt+1 (3 heads each)
            for proj, qofs in (("q", 0), ("k", 768)):
                ps = psA.tile([128, 1024], FP32, tag="psA", name="ps")
                for qp in range(2):
                    q6 = pt * 2 + qp
                    for eh in range(2):
                        nc.tensor.matmul(
                            ps[0:96, qp * TOK:(qp + 1) * TOK],
                            wqk16[:, eh * 1536 + qofs + q6 * 128:
                                  eh * 1536 + qofs + q6 * 128 + 96],
                            xr2[:, eh * TOK:(eh + 1) * TOK] if proj == "q"
                            else xc2[:, eh * TOK:(eh + 1) * TOK],
                            start=(eh == 0), stop=(eh == 1))
                qt = big_p.tile([96, 2 * TOK], FP16, tag=f"{proj}T{pt}",
                                name=f"{proj}T{pt}")
                bal.copy(qt[:], ps[0:96, :], 1024)
                quads[(proj, pt)] = qt

        vhats = {}
        vh_tiles = {}
        for b in range(BL):
            vh = big_p.tile([128, 17 * H], FP16, tag=f"vhat{b}",
                            name=f"vhat{b}")
            vh_tiles[b] = vh
            nc.gpsimd.memset(
                vh[:].rearrange("p (h x) -> p h x", h=H)[:, :, 16:17], 1.0)

        def emit_vhat(b):
            vh = vh_tiles[b]
            vh3 = vh[:].rearrange("p (h x) -> p h x", h=H)
            ps = psA.tile([128, 1024], FP32, tag="psA", name="ps")
            for eh in range(2):
                nc.tensor.matmul(
                    ps[:, 0:E],
                    xc2[:, eh * TOK + b * 128:eh * TOK + (b + 1) * 128],
                    wv16[:, eh * E:(eh + 1) * E],
                    start=(eh == 0), stop=(eh == 1))
            bal.copy(vh3[:, :, 0:16],
                     ps[:, 0:E].rearrange("p (h x) -> p h x", h=H), E)
            vhats[b] = vh

        # proj runs 2 mh ahead of dots; vhat interleaved to fill gaps
        emit_proj(0)
        emit_proj(1)
        for mh in range(4):
            for hp2 in range(2):   # dots for heads 4mh .. 4mh+3
                ps = psA.tile([128, 1024], FP32, tag="psA", name="ps")
                for hh in range(2):
                    h = mh * 4 + hp2 * 2 + hh
                    for b in range(BL):
                        nc.tensor.matmul(
                            ps[:, hh * 512 + b * 128:hh * 512 + (b + 1) * 128],
                            headT("q", h)[:, b * 128:(b + 1) * 128],
                            headT("k", h)[:, b * 128:(b + 1) * 128])
                hbase = mh * 4 + hp2 * 2
                bal.copy(x4all[:, hbase * 512:(hbase + 2) * 512], ps[:], 1024)
            # stage1 for b0/b1 only (first 256 cols of each h-block)
            nc.sync.dma_start(
                scr01s[mh][:],
                x4all[:].rearrange("r (h x) -> r h x", h=H)
                [:, mh * 4:(mh + 1) * 4, 0:256])
            # stage2 quad DMA for b0 only (critical path); b1 deferred
            nc.sync.dma_start(
                rhs0[mh * 4:(mh + 1) * 4, :].rearrange(
                    "h (r c) -> h r c", c=C),
                s01v[mh][:, :, 0, :].rearrange("r h c -> h r c"))
            if mh == 1:
                emit_proj(2)

        for b in range(BL):
            emit_vhat(b)

        # ---- per-batch MLP + softmax + AV
        fouts = [fout_p.tile([128, H * D], FP32, tag=f"fo{b}", name=f"fo{b}")
                 for b in range(BL)]

        def assemble(b):
            rhs = rhs_p.tile([17, PTS], FP16, tag="rhs", name="rhs")
            # dots rows: dst (h; r, c) <- scratch [h, r, b, c] slice
            for q in range(4):
                nc.sync.dma_start(
                    rhs[q * 4:(q + 1) * 4, :].rearrange(
                        "h (r c) -> h r c", c=C),
                    s23v[q][:, :, b - 2, :].rearrange("r h c -> h r c"))
            nc.sync.dma_start(rhs[16:17, :], cost_d[b:b + 1, :])
            return rhs

        def make_tail(b, half, wsbs):
            # AV + normalize for (b, half), deferred into the next half's
            # pair loop so the PE never stalls on the ACT-queued exp
            def tail():
                psa = ps2_p.tile([128, 17 * 8], FP32, tag="ps2", name="psa")
                for grp in range(2):
                    w4 = wsbs[grp][:].rearrange("p (r h) -> p r h", h=8)
                    for hl in range(8):
                        h = half * 8 + hl
                        nc.tensor.matmul(
                            psa[grp * 64:(grp + 1) * 64,
                                hl * 17:hl * 17 + 17],
                            w4[:, :, hl],
                            vhats[b][:, h * 17:(h + 1) * 17])
                psa3 = psa[:].rearrange("p (x y) -> p x y", x=8)
                rec = small_p.tile([128, 8], FP32, tag="rec")
                nc.vector.reciprocal(rec[:], psa3[:, :, 16])
                bal.charge("D", 8)
                recb = rec[:].rearrange(
                    "p (h o) -> p h o", o=1).broadcast_to([128, 8, 16])
                nc.vector.tensor_tensor(
                    fouts[b][:, half * 128:(half + 1) * 128].rearrange(
                        "p (h x) -> p h x", h=8),
                    psa3[:, :, 0:16], recb, ALU.mult)
                bal.charge("D", 128)
                if half == 1:
                    nc.sync.dma_start(out_d[b], fouts[b][:])
            return tail

        # deferred: b1's stage2 quads and b2/b3 stage1 — emitted after
        # b0's full chain so their transfers never contend with it
        for q in range(4):
            nc.sync.dma_start(
                rhs1[q * 4:(q + 1) * 4, :].rearrange(
                    "h (r c) -> h r c", c=C),
                s01v[q][:, :, 1, :].rearrange("r h c -> h r c"))
        for q in range(4):
            nc.sync.dma_start(
                scr23s[q][:],
                x4all[:].rearrange("r (h x) -> r h x", h=H)
                [:, q * 4:(q + 1) * 4, 256:512])

        rhss = {0: rhs0, 1: rhs1}
        prev_tail = None
        for b in range(BL):
            if b + 2 < BL:
                rhss[b + 2] = assemble(b + 2)
            rhs = rhss.pop(b)
            for half in range(2):
                w1h = w1sb[:, half * 128:(half + 1) * 128]
                w2h = w2sb[:, half * 8:(half + 1) * 8]
                bch = bc2[:, half:half + 1]
                ps2s = []
                pend = []   # (rr, pair) with layer2 not yet emitted

                def emit_l2(rr, pair, ps2s=ps2s, w2h=w2h):
                    if pair % 8 == 0:
                        ps2t = ps2_p.tile([128, 512], FP32, tag="ps2",
                                          name="ps2t")
                        ps2s.append(ps2t)
                    for s in range(8):
                        rloc = (pair % 8) * 8 + s
                        nc.tensor.matmul(
                            ps2s[-1][:, rloc * 8:rloc * 8 + 8],
                            rr[:, s * 128:(s + 1) * 128], w2h)

                wsbs = []
                for pair in range(16):
                    ps1 = psA.tile([128, 1024], FP32, tag="psA")
                    for k in range(2):
                        ck = pair * 2 + k
                        nc.tensor.matmul(
                            ps1[:, k * 512:(k + 1) * 512], w1h,
                            rhs[:, ck * 512:(ck + 1) * 512])
                    rr = rr_p.tile([128, 1024], FP16, tag="rr")
                    if pair % 2 == 0:
                        nc.vector.tensor_scalar(rr[:], ps1[:], bch, 0.0,
                                                ALU.add, ALU.max)
                        bal.charge("D", 1024)
                    else:
                        nc.scalar.activation(rr[:], ps1[:], AF.Relu, bias=bch)
                        bal.charge("A", 1024)
                    pend.append((rr, pair))
                    if len(pend) > 4:
                        emit_l2(*pend.pop(0))
                    if pair == 4 and prev_tail is not None:
                        prev_tail()
                        prev_tail = None
                    # grp0 fully written once l2(p7) emitted -> exp it early
                    if pend and pend[0][1] == 8 and len(ps2s) == 1:
                        wsb = wsb_p.tile([128, 512], FP16, tag="wsb")
                        nc.scalar.activation(wsb[:], ps2s[0][:], AF.Exp)
                        bal.charge("A", 512)
                        wsbs.append(wsb)
                while pend:
                    emit_l2(*pend.pop(0))
                wsb = wsb_p.tile([128, 512], FP16, tag="wsb")
                nc.scalar.activation(wsb[:], ps2s[1][:], AF.Exp)
                bal.charge("A", 512)
                wsbs.append(wsb)
                prev_tail = make_tail(b, half, wsbs)

        prev_tail()

    nc.compile()
    return nc


_cache = {}


def _prep(inputs):
    row_emb = np.asarray(inputs["row_emb"], dtype=np.float32)
    col_emb = np.asarray(inputs["col_emb"], dtype=np.float32)
    cost_mat = np.asarray(inputs["cost_mat"], dtype=np.float32)
    Wq = np.asarray(inputs["Wq"], dtype=np.float32) / np.sqrt(D)
    Wk = np.asarray(inputs["Wk"], dtype=np.float32)
    Wv = np.asarray(inputs["Wv"], dtype=np.float32)
    m1w = np.asarray(inputs["mix1_weight"], dtype=np.float32)
    m1b = np.asarray(inputs["mix1_bias"], dtype=np.float32)
    m2w = np.asarray(inputs["mix2_weight"], dtype=np.float32)

    a1, c1 = m1w[:, 0, :], m1w[:, 1, :]
    w2 = m2w[:, :, 0]

    # layer1 stationary [17, 256]: col (half*128 + hl*16 + m)
    w1l = np.zeros((17, 256), dtype=np.float16)
    w2l = np.zeros((128, 16), dtype=np.float16)
    bcol2 = np.zeros((128, 2), dtype=np.float32)
    for h in range(H):
        half, hl = h // 8, h % 8
        for m in range(MS):
            col = half * 128 + hl * 16 + m
            w1l[h, col] = a1[h, m]
            w1l[16, col] = c1[h, m]
            w2l[hl * 16 + m, half * 8 + hl] = w2[h, m]
            bcol2[hl * 16 + m, half] = m1b[h, m]

    # padded q/k weights: head h -> 32-col slot (h%4)*32 within quad h//4
    def pad_qk(w):
        wp = np.zeros((E, 768), dtype=np.float16)
        w4 = w.reshape(E, H, D)
        for h in range(H):
            q6, sl = h // 3, h % 3
            wp[:, q6 * 128 + sl * 32:q6 * 128 + sl * 32 + 16] = w4[:, h, :]
        return wp

    def fold_eh(a):
        # [E, X] -> [128, (eh, X)]: row eh*128+p -> partition p, col-block eh
        x = a.reshape(2, 128, a.shape[1]).transpose(1, 0, 2)
        return np.ascontiguousarray(x.reshape(128, -1).astype(np.float16))

    wqk = fold_eh(np.concatenate([pad_qk(Wq), pad_qk(Wk)], axis=1))
    wv = fold_eh(Wv)

    per_core = []
    for i in range(NCORES):
        sl = slice(i * BL, (i + 1) * BL)
        xr = row_emb[sl].reshape(TOK, E).T
        xc = col_emb[sl].reshape(TOK, E).T
        per_core.append({
            "xrT": fold_eh(xr),
            "xcT": fold_eh(xc),
            "wqk": wqk,
            "wv": wv,
            "w1l": w1l,
            "w2l": w2l,
            "bcol2": bcol2,
            "cost16": cost_mat[sl].reshape(BL, PTS).astype(np.float16),
        })
    return per_core


def kernel(**inputs):
    if "nc" not in _cache:
        _cache["nc"] = build_kernel()
    nc = _cache["nc"]
    in_maps = _prep(inputs)
    res = run_bass_kernel_spmd(nc, in_maps, list(range(NCORES)))
    out = np.concatenate([res.results[i]["out"] for i in range(NCORES)],
                         axis=0)
    return out.astype(np.float32)



# revision 36
# speedup vs baseline: 1.0557x; 1.0557x over previous
"""MixedScoreMultiHeadAttention TRN2 kernel, v3.

Data-parallel over batch: 32 batches -> 8 cores x 4 batches (BL=4).

Key design points (vs v2 baseline at 120.3us -> 113.5us):
  - layer1 (the dominant matmul) runs as fp8e4 DoubleRow (0.5 cyc/row):
    scores+w1 packed into [9, 2, .] group layout; halves PE time and
    takes the PE off the critical path entirely.
  - proj: 4 tiles/side [128, 512]; head h -> tile h%4, 32-slot h//4.
    Heads sharing a slot sit in different tiles so every dots PSUM BANK
    receives a single tile_position (mixing row-positions within one
    bank crashes the hardware path).
  - dots emitted b-major; batch 0's rhs assembles earliest. Scores
    bounce via DRAM fp8 (x4all -> scr -> rhs [9, 2*16384]).
  - score rows: g0 = heads 0-7 + cost(p8), g1 = heads 8-15 + cost-dup
    (zero weights, keeps the pad row finite without a 16K memset).
  - DVE/ACT evacuation is the true bottleneck (~85us busy each):
    greedy-balanced relu/copy/exp split with calibrated costs.
  - PSUM: psA 3x[128,1024] rotation (the 3rd slot removes a ~500ns
    slot round-trip per relu turn); layer2 psum [128,1024] -> one exp
    per half; AV borrows a psA slot once per half (tail at t==12 so
    its readers drain before the slot is needed).
  - layer2 pend queue crosses half boundaries (leftover l2 matmuls
    interleave AFTER the next half's L1 fills, avoiding PE wait-queue
    head-of-line blocking); ps2 tile created lazily at first l2.
  - warmup: a single tiny matmul at t~0 starts the p-state ramp clock
    (full PE speed from ~3.4us, just as the projection weights land).
  - final half: exp split in two + AV split by r-halves to shorten the
    drain chain.
"""
import sys

sys.path.insert(0, "/opt/trn_rl_repo")

import numpy as np
from contextlib import ExitStack

import concourse.bass as bass
import concourse.mybir as mybir
import concourse.tile as tile
from concourse import bacc
from concourse.bass_utils import run_bass_kernel_spmd

B, R, C, E, H, D, MS = 32, 128, 128, 256, 16, 16, 16
NCORES = 8
BL = B // NCORES   # 4 batches per core
TOK = BL * R       # 512 tokens per core per side
PTS = R * C        # 16384 points per batch

FP32 = mybir.dt.float32
FP16 = mybir.dt.float16
FP8 = mybir.dt.float8e4
AF = mybir.ActivationFunctionType
ALU = mybir.AluOpType
DR = mybir.MatmulPerfMode.DoubleRow


class Balancer:
    """Greedy DVE/ACT load balancing for PSUM-exit elementwise ops."""

    def __init__(self, nc):
        self.nc = nc
        self.load = {"D": 0.0, "A": 0.0}

    @staticmethod
    def _cost(eng, n, psum_src):
        if eng == "D":
            return n * 1.0417 + (125.0 if psum_src else 60.0)
        return n * 0.8333 + 185.0

    def _pick(self, n, psum_src, forbid=()):
        e = min((x for x in "DA" if x not in forbid),
                key=lambda x: self.load[x])
        self.load[e] += self._cost(e, n, psum_src)
        return e

    def charge(self, eng, n, psum_src=True):
        self.load[eng] += self._cost(eng, n, psum_src)

    def copy(self, out, in_, n, forbid=()):
        if self._pick(n, True, forbid) == "D":
            self.nc.vector.tensor_copy(out, in_)
        else:
            self.nc.scalar.copy(out, in_)

    def relu(self, out, in_, bias_ap, n, forbid=()):
        if self._pick(n, True, forbid) == "D":
            self.nc.vector.tensor_scalar(out, in_, bias_ap, 0.0,
                                         ALU.add, ALU.max)
        else:
            self.nc.scalar.activation(out, in_, AF.Relu, bias=bias_ap)


def build_kernel():
    nc = bacc.Bacc("TRN2", target_bir_lowering=False, debug=False,
                   num_devices=NCORES)

    xr_d = nc.dram_tensor("xrT", [128, 2 * TOK], FP16,
                          kind="ExternalInput").ap()
    xc_d = nc.dram_tensor("xcT", [128, 2 * TOK], FP16,
                          kind="ExternalInput").ap()
    # q/k weights: per eh block [q 512 | k 512], natural head-major order
    wqk_d = nc.dram_tensor("wqk", [128, 2 * 2 * 512], FP16,
                           kind="ExternalInput").ap()
    wv_d = nc.dram_tensor("wv", [128, 2 * E], FP16,
                          kind="ExternalInput").ap()
    # layer1 stationary, DoubleRow grouped: [9, (half, g, 128)] fp8
    w1_d = nc.dram_tensor("w1l8", [9, 512], FP8, kind="ExternalInput").ap()
    w2_d = nc.dram_tensor("w2l", [128, 16], FP16, kind="ExternalInput").ap()
    bc_d = nc.dram_tensor("bcol2", [128, 2], FP32, kind="ExternalInput").ap()
    cost_d = nc.dram_tensor("cost8", [BL, PTS], FP8,
                            kind="ExternalInput").ap()
    out_d = nc.dram_tensor("out", [BL, R, H * D], FP32,
                           kind="ExternalOutput").ap()
    # DRAM bounce for dot-score transposition: [r, (b, h, c)] fp8
    scr_d = nc.dram_tensor("scr", [128, BL * H * C], FP8,
                           kind="Internal").ap()

    with tile.TileContext(nc) as tc, ExitStack() as ctx:
        const_p = ctx.enter_context(tc.tile_pool(name="const", bufs=1))
        big_p = ctx.enter_context(tc.tile_pool(name="big", bufs=1))
        rhs_p = ctx.enter_context(tc.tile_pool(name="rhs", bufs=2))
        rr_p = ctx.enter_context(tc.tile_pool(name="rr", bufs=8))
        wsb_p = ctx.enter_context(tc.tile_pool(name="wsb", bufs=2))
        fout_p = ctx.enter_context(tc.tile_pool(name="fout", bufs=1))
        small_p = ctx.enter_context(tc.tile_pool(name="small", bufs=4))
        psA = ctx.enter_context(
            tc.tile_pool(name="psA", bufs=3, space="PSUM"))   # [128,1024] x3
        ps2_p = ctx.enter_context(
            tc.tile_pool(name="ps2", bufs=1, space="PSUM"))   # [128,1024] x1

        bal = Balancer(nc)

        # ---- PE warmup: one tiny matmul starts the p-state ramp clock
        # (pe_busy_start pins to the first matmul; after +3us wall time the
        # PE runs at full speed, so the clock just needs to start early)
        zwarm = const_p.tile([1, 1], FP16)
        nc.gpsimd.memset(zwarm[:], 0.0)
        wps = ps2_p.tile([128, 1024], FP32, tag="ps2", name="wps")
        nc.tensor.matmul(wps[0:1, 0:1], zwarm[:], zwarm[:])

        # ---- const loads (q-halves of wqk + xr first: proj q needs them)
        xr2 = const_p.tile([128, 2 * TOK], FP16)   # [p, (eh, tok)]
        xc2 = const_p.tile([128, 2 * TOK], FP16)
        wqk16 = const_p.tile([128, 2 * 2 * 512], FP16)
        wv16 = const_p.tile([128, 2 * E], FP16)
        nc.sync.dma_start(xr2[:], xr_d[:])
        nc.sync.dma_start(wqk16[:, 0:512], wqk_d[:, 0:512])
        nc.sync.dma_start(wqk16[:, 1024:1536], wqk_d[:, 1024:1536])
        nc.sync.dma_start(xc2[:], xc_d[:])
        nc.sync.dma_start(wqk16[:, 512:1024], wqk_d[:, 512:1024])
        nc.sync.dma_start(wqk16[:, 1536:2048], wqk_d[:, 1536:2048])
        w1sb = const_p.tile([9, 512], FP8)
        nc.gpsimd.dma_start(w1sb[:], w1_d[:])
        w2sb = const_p.tile([128, 16], FP16)
        nc.gpsimd.dma_start(w2sb[:], w2_d[:])
        bc2 = const_p.tile([128, 2], FP32)
        nc.gpsimd.dma_start(bc2[:], bc_d[:])
        nc.gpsimd.dma_start(wv16[:], wv_d[:])

        # ---- proj: 4 tiles per side [128, 512]; head h -> tile h%4 at
        # 32-slot h//4 (16 rows used + 16 zero). Heads sharing a slot land
        # in different tiles, so each dots PSUM bank (4 heads) sees a
        # single tile_position.
        quads = {}

        def emit_proj(side, ofs, ft):
            x = xr2 if side == "q" else xc2
            ps = psA.tile([128, 1024], FP32, tag="psA", name="ps")
            for eh in range(2):
                nc.tensor.matmul(
                    ps[:, 0:TOK],
                    wqk16[:, eh * 1024 + ofs + ft * 128:
                          eh * 1024 + ofs + ft * 128 + 128],
                    x[:, eh * TOK:(eh + 1) * TOK],
                    start=(eh == 0), stop=(eh == 1))
            qt = big_p.tile([128, TOK], FP16, tag=f"{side}T{ft}",
                            name=f"{side}T{ft}")
            bal.copy(qt[:], ps[:, 0:TOK], TOK)
            quads[(side, ft)] = qt

        for ft in range(4):
            emit_proj("q", 0, ft)
        for ft in range(4):
            emit_proj("k", 512, ft)

        # ---- vhat tiles (ones column pre-set for softmax denominator)
        vhats = {}
        vh_tiles = {}
        for b in range(BL):
            vh = big_p.tile([128, 17 * H], FP16, tag=f"vhat{b}",
                            name=f"vhat{b}")
            vh_tiles[b] = vh
            nc.gpsimd.memset(
                vh[:].rearrange("p (h x) -> p h x", h=H)[:, :, 16:17], 1.0)

        def emit_vhat(b):
            vh = vh_tiles[b]
            vh3 = vh[:].rearrange("p (h x) -> p h x", h=H)
            ps = psA.tile([128, 1024], FP32, tag="psA", name="ps")
            for eh in range(2):
                nc.tensor.matmul(
                    ps[:, 0:E],
                    xc2[:, eh * TOK + b * 128:eh * TOK + (b + 1) * 128],
                    wv16[:, eh * E:(eh + 1) * E],
                    start=(eh == 0), stop=(eh == 1))
            bal.copy(vh3[:, :, 0:16],
                     ps[:, 0:E].rearrange("p (h x) -> p h x", h=H), E)
            vhats[b] = vh

        # ---- dots, b-major: per (b, hh) one psum [r, (h8, c)] -> x4all fp8
        x4all = big_p.tile([128, BL * H * C], FP8, name="x4all")

        def emit_dots(b, hh):
            ps = psA.tile([128, 1024], FP32, tag="psA", name="ps")
            for i in range(8):
                h = hh * 8 + i
                base = (h // 4) * 32
                qt = quads[("q", h % 4)]
                kt = quads[("k", h % 4)]
                nc.tensor.matmul(
                    ps[:, i * 128:(i + 1) * 128],
                    qt[base:base + 16, b * 128:(b + 1) * 128],
                    kt[base:base + 16, b * 128:(b + 1) * 128],
                    tile_position=(base, 0))
            co = b * H * C + hh * 1024
            if b <= 1:
                # split across both engines: early evacs gate the pipeline
                bal.copy(x4all[:, co:co + 512], ps[:, 0:512], 512)
                bal.copy(x4all[:, co + 512:co + 1024], ps[:, 512:1024], 512)
            else:
                bal.copy(x4all[:, co:co + 1024], ps[:], 1024)
            nc.sync.dma_start(scr_d[:, co:co + 1024], x4all[:, co:co + 1024])

        # ---- rhs assembly: [9, (g, pts)] fp8 per batch
        scr_v = scr_d[:].rearrange("r (b h c) -> r b h c", b=BL, h=H)

        def assemble(b):
            rhs = rhs_p.tile([9, 2 * PTS], FP8, tag="rhs", name=f"rhs{b}")
            # g0 rows 0..7 = heads 0..7 (depends only on stage1 of hh=0)
            nc.sync.dma_start(
                rhs[0:8, 0:PTS].rearrange("p (r c) -> p r c", c=C),
                scr_v[:, b, 0:8, :].rearrange("r h c -> h r c"))
            # g1 rows 0..7 = heads 8..15 (depends only on stage1 of hh=1)
            nc.sync.dma_start(
                rhs[0:8, PTS:2 * PTS].rearrange("p (r c) -> p r c", c=C),
                scr_v[:, b, 8:16, :].rearrange("r h c -> h r c"))
            # g0 p8 = cost (real); g1 p8 = cost dup (zero weight, finite)
            nc.gpsimd.dma_start(rhs[8:9, 0:PTS], cost_d[b:b + 1, :])
            nc.gpsimd.dma_start(rhs[8:9, PTS:2 * PTS], cost_d[b:b + 1, :])
            return rhs

        rhss = {}
        for b in range(BL):
            emit_dots(b, 0)
            emit_dots(b, 1)
            if b == 0:
                rhss[0] = assemble(0)
                emit_vhat(0)
            if b == 1:
                rhss[1] = assemble(1)
                emit_vhat(1)
            if b == 2:
                emit_vhat(2)
            if b == 3:
                emit_vhat(3)

        # ---- per-batch MLP + softmax + AV
        fouts = [fout_p.tile([128, H * D], FP32, tag=f"fo{b}", name=f"fo{b}")
                 for b in range(BL)]
        w1v = w1sb[:].rearrange("p (h g m) -> p h g m", h=2, g=2)

        def make_tail(b, half, wsb, split=False):
            def tail():
                # AV psum borrows a psA rotation slot briefly once per half
                pst = psA.tile([128, 1024], FP32, tag="psA", name="psav")
                psa = pst[:, 0:136]
                w4 = wsb[:].rearrange("p (r h) -> p h r", h=8)
                for hl in range(8):
                    h = half * 8 + hl
                    if split:
                        # two r-halves: part0 depends only on the early exp
                        nc.tensor.matmul(
                            psa[0:64, hl * 17:hl * 17 + 17],
                            w4[:, hl, 0:64],
                            vhats[b][:, h * 17:(h + 1) * 17])
                        nc.tensor.matmul(
                            psa[64:128, hl * 17:hl * 17 + 17],
                            w4[:, hl, 64:128],
                            vhats[b][:, h * 17:(h + 1) * 17],
                            tile_position=(0, 64))
                    else:
                        nc.tensor.matmul(
                            psa[:, hl * 17:hl * 17 + 17],
                            w4[:, hl, :],
                            vhats[b][:, h * 17:(h + 1) * 17])
                psa3 = psa.rearrange("p (x y) -> p x y", x=8)
                rec = small_p.tile([128, 8], FP32, tag="rec")
                nc.vector.reciprocal(rec[:], psa3[:, :, 16])
                bal.charge("D", 8)
                recb = rec[:].rearrange(
                    "p (h o) -> p h o", o=1).broadcast_to([128, 8, 16])
                nc.vector.tensor_tensor(
                    fouts[b][:, half * 128:(half + 1) * 128].rearrange(
                        "p (h x) -> p h x", h=8),
                    psa3[:, :, 0:16], recb, ALU.mult)
                bal.charge("D", 128)
                if half == 1:
                    nc.sync.dma_start(out_d[b], fouts[b][:])
            return tail

        prev_tail = None
        pend = []        # (rr, t, halfctx) layer2 not yet emitted
        halfctx = None   # mutable [ps2, w2h, n_emitted, b, half]

        def emit_l2(rr, t, hc):
            if hc[0] is None:
                hc[0] = ps2_p.tile([128, 1024], FP32, tag="ps2", name="ps2")
            ps2, w2h = hc[0], hc[1]
            for s in range(8):
                rloc = t * 8 + s
                nc.tensor.matmul(
                    ps2[:, rloc * 8:rloc * 8 + 8],
                    rr[:, s * 128:(s + 1) * 128], w2h)
            hc[2] += 1
            last = hc[3] == BL - 1 and hc[4] == 1
            if last and hc[2] == 8:
                # final half: split exp so the AV chain starts earlier
                wsb = wsb_p.tile([128, 1024], FP16, tag="wsb")
                hc.append(wsb)
                nc.scalar.activation(wsb[:, 0:512], ps2[:, 0:512], AF.Exp)
                bal.charge("A", 512)
            if hc[2] == 16:   # half fully reduced -> exp + schedule tail
                if last:
                    wsb = hc[5]
                    nc.scalar.activation(wsb[:, 512:1024], ps2[:, 512:1024],
                                         AF.Exp)
                    bal.charge("A", 512)
                    return make_tail(hc[3], hc[4], wsb, split=True)
                wsb = wsb_p.tile([128, 1024], FP16, tag="wsb")
                nc.scalar.activation(wsb[:], ps2[:], AF.Exp)
                bal.charge("A", 1024)
                return make_tail(hc[3], hc[4], wsb)
            return None

        for b in range(BL):
            if b + 2 < BL:
                rhss[b + 2] = assemble(b + 2)
            rhs = rhss.pop(b)
            rhs3 = rhs[:].rearrange("p (g n) -> p g n", g=2)
            for half in range(2):
                w1h = w1v[:, half]            # [9, 2, 128]
                w2h = w2sb[:, half * 8:(half + 1) * 8]
                bch = bc2[:, half:half + 1]
                halfctx = [None, w2h, 0, b, half]
                for t in range(16):   # 16 tiles of 1024 points per half
                    ps1 = psA.tile([128, 1024], FP32, tag="psA")
                    for k4 in range(2):
                        n0 = t * 1024 + k4 * 512
                        nc.tensor.matmul(
                            ps1[:, k4 * 512:(k4 + 1) * 512], w1h,
                            rhs3[:, :, n0:n0 + 512], perf_mode=DR)
                    rr = rr_p.tile([128, 1024], FP16, tag="rr")
                    bal.relu(rr[:], ps1[:], bch, 1024)
                    pend.append((rr, t, halfctx))
                    if len(pend) > 4:
                        nt = emit_l2(*pend.pop(0))
                        if nt is not None:
                            prev_tail = nt
                    if t == 12 and prev_tail is not None:
                        prev_tail()
                        prev_tail = None

        while pend:
            nt = emit_l2(*pend.pop(0))
            if nt is not None:
                prev_tail = nt
        prev_tail()

    nc.compile()
    return nc


_cache = {}


def _prep(inputs):
    import ml_dtypes
    FP8NP = ml_dtypes.float8_e4m3fn

    row_emb = np.asarray(inputs["row_emb"], dtype=np.float32)
    col_emb = np.asarray(inputs["col_emb"], dtype=np.float32)
    cost_mat = np.asarray(inputs["cost_mat"], dtype=np.float32)
    Wq = np.asarray(inputs["Wq"], dtype=np.float32) / np.sqrt(D)
    Wk = np.asarray(inputs["Wk"], dtype=np.float32)
    Wv = np.asarray(inputs["Wv"], dtype=np.float32)
    m1w = np.asarray(inputs["mix1_weight"], dtype=np.float32)
    m1b = np.asarray(inputs["mix1_bias"], dtype=np.float32)
    m2w = np.asarray(inputs["mix2_weight"], dtype=np.float32)

    a1, c1 = m1w[:, 0, :], m1w[:, 1, :]
    w2 = m2w[:, :, 0]

    # layer1 stationary fp8, DoubleRow grouped: w1l8[p, (half, g, col)]
    # g0: heads 0..7 at p0..7, cost at p8 (c1 weights)
    # g1: heads 8..15 at p0..7, cost-dup at p8 (zero weights)
    w1l8 = np.zeros((9, 2, 2, 128), dtype=np.float32)
    w2l = np.zeros((128, 16), dtype=np.float16)
    bcol2 = np.zeros((128, 2), dtype=np.float32)
    for h in range(H):
        half, hl = h // 8, h % 8
        g, p = h // 8, h % 8
        for m in range(MS):
            col = hl * 16 + m
            w1l8[p, half, g, col] = a1[h, m]
            w1l8[8, half, 0, col] = c1[h, m]     # cost row -> (g0, p8)
            w2l[hl * 16 + m, half * 8 + hl] = w2[h, m]
            bcol2[hl * 16 + m, half] = m1b[h, m]
    w1l8 = w1l8.reshape(9, 512).astype(FP8NP)

    # q/k weights: head h -> tile h%4 (128-col block), slot h//4 (32 cols)
    def pad_qk(w):
        wp = np.zeros((E, 512), dtype=np.float16)
        w4 = w.reshape(E, H, D)
        for h in range(H):
            base = (h % 4) * 128 + (h // 4) * 32
            wp[:, base:base + 16] = w4[:, h, :]
        return wp

    def fold_eh(a):
        # [E, X] -> [128, (eh, X)]: row eh*128+p -> partition p, col-block eh
        x = a.reshape(2, 128, a.shape[1]).transpose(1, 0, 2)
        return np.ascontiguousarray(x.reshape(128, -1).astype(np.float16))

    wqk = fold_eh(np.concatenate([pad_qk(Wq), pad_qk(Wk)], axis=1))
    wv = fold_eh(Wv)

    per_core = []
    for i in range(NCORES):
        sl = slice(i * BL, (i + 1) * BL)
        xr = row_emb[sl].reshape(TOK, E).T
        xc = col_emb[sl].reshape(TOK, E).T
        per_core.append({
            "xrT": fold_eh(xr),
            "xcT": fold_eh(xc),
            "wqk": wqk,
            "wv": wv,
            "w1l8": w1l8,
            "w2l": w2l,
            "bcol2": bcol2,
            "cost8": cost_mat[sl].reshape(BL, PTS).astype(FP8NP),
        })
    return per_core


def kernel(**inputs):
    if "nc" not in _cache:
        _cache["nc"] = build_kernel()
    nc = _cache["nc"]
    in_maps = _prep(inputs)
    res = run_bass_kernel_spmd(nc, in_maps, list(range(NCORES)))
    out = np.concatenate([res.results[i]["out"] for i in range(NCORES)],
                         axis=0)
    return out.astype(np.float32)


# revision 37
# speedup vs baseline: 1.0595x; 1.0036x over previous
"""MixedScoreMultiHeadAttention TRN2 kernel, v3.

Data-parallel over batch: 32 batches -> 8 cores x 4 batches (BL=4).

Key design points (vs v2 baseline at 120.3us -> 113.5us):
  - layer1 (the dominant matmul) runs as fp8e4 DoubleRow (0.5 cyc/row):
    scores+w1 packed into [9, 2, .] group layout; halves PE time and
    takes the PE off the critical path entirely.
  - proj: 4 tiles/side [128, 512]; head h -> tile h%4, 32-slot h//4.
    Heads sharing a slot sit in different tiles so every dots PSUM BANK
    receives a single tile_position (mixing row-positions within one
    bank crashes the hardware path).
  - dots emitted b-major; batch 0's rhs assembles earliest. Scores
    bounce via DRAM fp8 (x4all -> scr -> rhs [9, 2*16384]).
  - score rows: g0 = heads 0-7 + cost(p8), g1 = heads 8-15 + cost-dup
    (zero weights, keeps the pad row finite without a 16K memset).
  - DVE/ACT evacuation is the true bottleneck (~85us busy each):
    greedy-balanced relu/copy/exp split with calibrated costs.
  - PSUM: psA 3x[128,1024] rotation (the 3rd slot removes a ~500ns
    slot round-trip per relu turn); layer2 psum [128,1024] -> one exp
    per half; AV borrows a psA slot once per half (tail at t==12 so
    its readers drain before the slot is needed).
  - layer2 pend queue crosses half boundaries (leftover l2 matmuls
    interleave AFTER the next half's L1 fills, avoiding PE wait-queue
    head-of-line blocking); ps2 tile created lazily at first l2.
  - warmup: a single tiny matmul at t~0 starts the p-state ramp clock
    (full PE speed from ~3.4us, just as the projection weights land).
  - final half: exp split in two + AV split by r-halves to shorten the
    drain chain.
"""
import sys

sys.path.insert(0, "/opt/trn_rl_repo")

import numpy as np
from contextlib import ExitStack

import concourse.bass as bass
import concourse.mybir as mybir
import concourse.tile as tile
from concourse import bacc
from concourse.bass_utils import run_bass_kernel_spmd

B, R, C, E, H, D, MS = 32, 128, 128, 256, 16, 16, 16
NCORES = 8
BL = B // NCORES   # 4 batches per core
TOK = BL * R       # 512 tokens per core per side
PTS = R * C        # 16384 points per batch

FP32 = mybir.dt.float32
FP16 = mybir.dt.float16
FP8 = mybir.dt.float8e4
AF = mybir.ActivationFunctionType
ALU = mybir.AluOpType
DR = mybir.MatmulPerfMode.DoubleRow


class Balancer:
    """Greedy DVE/ACT load balancing for PSUM-exit elementwise ops."""

    def __init__(self, nc):
        self.nc = nc
        self.load = {"D": 0.0, "A": 0.0}

    @staticmethod
    def _cost(eng, n, psum_src):
        if eng == "D":
            return n * 1.0417 + (125.0 if psum_src else 60.0)
        return n * 0.8333 + 185.0

    def _pick(self, n, psum_src, forbid=()):
        e = min((x for x in "DA" if x not in forbid),
                key=lambda x: self.load[x])
        self.load[e] += self._cost(e, n, psum_src)
        return e

    def charge(self, eng, n, psum_src=True):
        self.load[eng] += self._cost(eng, n, psum_src)

    def copy(self, out, in_, n, forbid=()):
        if self._pick(n, True, forbid) == "D":
            self.nc.vector.tensor_copy(out, in_)
        else:
            self.nc.scalar.copy(out, in_)

    def relu(self, out, in_, bias_ap, n, forbid=()):
        if self._pick(n, True, forbid) == "D":
            self.nc.vector.tensor_scalar(out, in_, bias_ap, 0.0,
                                         ALU.add, ALU.max)
        else:
            self.nc.scalar.activation(out, in_, AF.Relu, bias=bias_ap)


def build_kernel():
    nc = bacc.Bacc("TRN2", target_bir_lowering=False, debug=False,
                   num_devices=NCORES)

    xr_d = nc.dram_tensor("xrT", [128, 2 * TOK], FP16,
                          kind="ExternalInput").ap()
    xc_d = nc.dram_tensor("xcT", [128, 2 * TOK], FP16,
                          kind="ExternalInput").ap()
    # q/k weights: per eh block [q 512 | k 512], natural head-major order
    wqk_d = nc.dram_tensor("wqk", [128, 2 * 2 * 512], FP16,
                           kind="ExternalInput").ap()
    wv_d = nc.dram_tensor("wv", [128, 2 * E], FP16,
                          kind="ExternalInput").ap()
    # layer1 stationary, DoubleRow grouped: [9, (half, g, 128)] fp8
    w1_d = nc.dram_tensor("w1l8", [9, 512], FP8, kind="ExternalInput").ap()
    w2_d = nc.dram_tensor("w2l", [128, 16], FP16, kind="ExternalInput").ap()
    bc_d = nc.dram_tensor("bcol2", [128, 2], FP32, kind="ExternalInput").ap()
    cost_d = nc.dram_tensor("cost8", [BL, PTS], FP8,
                            kind="ExternalInput").ap()
    out_d = nc.dram_tensor("out", [BL, R, H * D], FP32,
                           kind="ExternalOutput").ap()
    # DRAM bounce for dot-score transposition: [r, (b, h, c)] fp8
    scr_d = nc.dram_tensor("scr", [128, BL * H * C], FP8,
                           kind="Internal").ap()

    with tile.TileContext(nc) as tc, ExitStack() as ctx:
        const_p = ctx.enter_context(tc.tile_pool(name="const", bufs=1))
        big_p = ctx.enter_context(tc.tile_pool(name="big", bufs=1))
        rhs_p = ctx.enter_context(tc.tile_pool(name="rhs", bufs=2))
        rr_p = ctx.enter_context(tc.tile_pool(name="rr", bufs=8))
        wsb_p = ctx.enter_context(tc.tile_pool(name="wsb", bufs=2))
        fout_p = ctx.enter_context(tc.tile_pool(name="fout", bufs=1))
        small_p = ctx.enter_context(tc.tile_pool(name="small", bufs=4))
        psA = ctx.enter_context(
            tc.tile_pool(name="psA", bufs=3, space="PSUM"))   # [128,1024] x3
        ps2_p = ctx.enter_context(
            tc.tile_pool(name="ps2", bufs=1, space="PSUM"))   # [128,1024] x1

        bal = Balancer(nc)

        # ---- PE warmup: one tiny matmul starts the p-state ramp clock
        # (pe_busy_start pins to the first matmul; after +3us wall time the
        # PE runs at full speed, so the clock just needs to start early)
        zwarm = const_p.tile([1, 1], FP16)
        nc.gpsimd.memset(zwarm[:], 0.0)
        wps = ps2_p.tile([128, 1024], FP32, tag="ps2", name="wps")
        nc.tensor.matmul(wps[0:1, 0:1], zwarm[:], zwarm[:])

        # ---- const loads (q-halves of wqk + xr first: proj q needs them)
        xr2 = const_p.tile([128, 2 * TOK], FP16)   # [p, (eh, tok)]
        xc2 = const_p.tile([128, 2 * TOK], FP16)
        wqk16 = const_p.tile([128, 2 * 2 * 512], FP16)
        wv16 = const_p.tile([128, 2 * E], FP16)
        nc.sync.dma_start(xr2[:], xr_d[:])
        nc.sync.dma_start(wqk16[:, 0:512], wqk_d[:, 0:512])
        nc.sync.dma_start(wqk16[:, 1024:1536], wqk_d[:, 1024:1536])
        nc.sync.dma_start(xc2[:], xc_d[:])
        nc.sync.dma_start(wqk16[:, 512:1024], wqk_d[:, 512:1024])
        nc.sync.dma_start(wqk16[:, 1536:2048], wqk_d[:, 1536:2048])
        w1sb = const_p.tile([9, 512], FP8)
        nc.gpsimd.dma_start(w1sb[:], w1_d[:])
        w2sb = const_p.tile([128, 16], FP16)
        nc.gpsimd.dma_start(w2sb[:], w2_d[:])
        bc2 = const_p.tile([128, 2], FP32)
        nc.gpsimd.dma_start(bc2[:], bc_d[:])
        nc.gpsimd.dma_start(wv16[:], wv_d[:])

        # ---- proj: 4 tiles per side [128, 512]; head h -> tile h%4 at
        # 32-slot h//4 (16 rows used + 16 zero). Heads sharing a slot land
        # in different tiles, so each dots PSUM bank (4 heads) sees a
        # single tile_position.
        quads = {}

        def emit_proj(side, ofs, ft):
            x = xr2 if side == "q" else xc2
            ps = psA.tile([128, 1024], FP32, tag="psA", name="ps")
            for eh in range(2):
                nc.tensor.matmul(
                    ps[:, 0:TOK],
                    wqk16[:, eh * 1024 + ofs + ft * 128:
                          eh * 1024 + ofs + ft * 128 + 128],
                    x[:, eh * TOK:(eh + 1) * TOK],
                    start=(eh == 0), stop=(eh == 1))
            qt = big_p.tile([128, TOK], FP16, tag=f"{side}T{ft}",
                            name=f"{side}T{ft}")
            bal.copy(qt[:], ps[:, 0:TOK], TOK)
            quads[(side, ft)] = qt

        for ft in range(4):
            emit_proj("q", 0, ft)
        for ft in range(4):
            emit_proj("k", 512, ft)

        # ---- vhat tiles (ones column pre-set for softmax denominator)
        vhats = {}
        vh_tiles = {}
        for b in range(BL):
            vh = big_p.tile([128, 17 * H], FP16, tag=f"vhat{b}",
                            name=f"vhat{b}")
            vh_tiles[b] = vh
            nc.gpsimd.memset(
                vh[:].rearrange("p (h x) -> p h x", h=H)[:, :, 16:17], 1.0)

        def emit_vhat(b):
            vh = vh_tiles[b]
            vh3 = vh[:].rearrange("p (h x) -> p h x", h=H)
            ps = psA.tile([128, 1024], FP32, tag="psA", name="ps")
            for eh in range(2):
                nc.tensor.matmul(
                    ps[:, 0:E],
                    xc2[:, eh * TOK + b * 128:eh * TOK + (b + 1) * 128],
                    wv16[:, eh * E:(eh + 1) * E],
                    start=(eh == 0), stop=(eh == 1))
            bal.copy(vh3[:, :, 0:16],
                     ps[:, 0:E].rearrange("p (h x) -> p h x", h=H), E)
            vhats[b] = vh

        # ---- dots, b-major: per (b, hh) one psum [r, (h8, c)] -> x4all fp8
        x4all = big_p.tile([128, BL * H * C], FP8, name="x4all")

        def emit_dots(b, hh):
            ps = psA.tile([128, 1024], FP32, tag="psA", name="ps")
            for i in range(8):
                h = hh * 8 + i
                base = (h // 4) * 32
                qt = quads[("q", h % 4)]
                kt = quads[("k", h % 4)]
                nc.tensor.matmul(
                    ps[:, i * 128:(i + 1) * 128],
                    qt[base:base + 16, b * 128:(b + 1) * 128],
                    kt[base:base + 16, b * 128:(b + 1) * 128],
                    tile_position=(base, 0))
            co = b * H * C + hh * 1024
            if b == 0:
                # split across both engines: rhs0 is the critical path
                bal.copy(x4all[:, co:co + 512], ps[:, 0:512], 512)
                bal.copy(x4all[:, co + 512:co + 1024], ps[:, 512:1024], 512)
            else:
                bal.copy(x4all[:, co:co + 1024], ps[:], 1024)
            nc.sync.dma_start(scr_d[:, co:co + 1024], x4all[:, co:co + 1024])

        # ---- rhs assembly: [9, (g, pts)] fp8 per batch
        scr_v = scr_d[:].rearrange("r (b h c) -> r b h c", b=BL, h=H)

        def assemble(b):
            rhs = rhs_p.tile([9, 2 * PTS], FP8, tag="rhs", name=f"rhs{b}")
            # g0 rows 0..7 = heads 0..7 (depends only on stage1 of hh=0)
            nc.sync.dma_start(
                rhs[0:8, 0:PTS].rearrange("p (r c) -> p r c", c=C),
                scr_v[:, b, 0:8, :].rearrange("r h c -> h r c"))
            # g1 rows 0..7 = heads 8..15 (depends only on stage1 of hh=1)
            nc.sync.dma_start(
                rhs[0:8, PTS:2 * PTS].rearrange("p (r c) -> p r c", c=C),
                scr_v[:, b, 8:16, :].rearrange("r h c -> h r c"))
            # g0 p8 = cost (real); g1 p8 = cost dup (zero weight, finite)
            nc.gpsimd.dma_start(rhs[8:9, 0:PTS], cost_d[b:b + 1, :])
            nc.gpsimd.dma_start(rhs[8:9, PTS:2 * PTS], cost_d[b:b + 1, :])
            return rhs

        rhss = {}
        for b in range(BL):
            emit_dots(b, 0)
            emit_dots(b, 1)
            if b == 0:
                rhss[0] = assemble(0)
                emit_vhat(0)
            if b == 1:
                rhss[1] = assemble(1)
                emit_vhat(1)
            if b == 2:
                emit_vhat(2)
            if b == 3:
                emit_vhat(3)

        # ---- per-batch MLP + softmax + AV
        fouts = [fout_p.tile([128, H * D], FP32, tag=f"fo{b}", name=f"fo{b}")
                 for b in range(BL)]
        w1v = w1sb[:].rearrange("p (h g m) -> p h g m", h=2, g=2)

        def make_tail(b, half, wsb, split=False):
            def tail():
                # AV psum borrows a psA rotation slot briefly once per half
                pst = psA.tile([128, 1024], FP32, tag="psA", name="psav")
                psa = pst[:, 0:136]
                w4 = wsb[:].rearrange("p (r h) -> p h r", h=8)
                for hl in range(8):
                    h = half * 8 + hl
                    if split:
                        # two r-halves: part0 depends only on the early exp
                        nc.tensor.matmul(
                            psa[0:64, hl * 17:hl * 17 + 17],
                            w4[:, hl, 0:64],
                            vhats[b][:, h * 17:(h + 1) * 17])
                        nc.tensor.matmul(
                            psa[64:128, hl * 17:hl * 17 + 17],
                            w4[:, hl, 64:128],
                            vhats[b][:, h * 17:(h + 1) * 17],
                            tile_position=(0, 64))
                    else:
                        nc.tensor.matmul(
                            psa[:, hl * 17:hl * 17 + 17],
                            w4[:, hl, :],
                            vhats[b][:, h * 17:(h + 1) * 17])
                psa3 = psa.rearrange("p (x y) -> p x y", x=8)
                rec = small_p.tile([128, 8], FP32, tag="rec")
                nc.vector.reciprocal(rec[:], psa3[:, :, 16])
                bal.charge("D", 8)
                recb = rec[:].rearrange(
                    "p (h o) -> p h o", o=1).broadcast_to([128, 8, 16])
                nc.vector.tensor_tensor(
                    fouts[b][:, half * 128:(half + 1) * 128].rearrange(
                        "p (h x) -> p h x", h=8),
                    psa3[:, :, 0:16], recb, ALU.mult)
                bal.charge("D", 128)
                if half == 1:
                    nc.sync.dma_start(out_d[b], fouts[b][:])
            return tail

        prev_tail = None
        pend = []        # (rr, t, halfctx) layer2 not yet emitted
        halfctx = None   # mutable [ps2, w2h, n_emitted, b, half]

        def emit_l2(rr, t, hc):
            if hc[0] is None:
                hc[0] = ps2_p.tile([128, 1024], FP32, tag="ps2", name="ps2")
            ps2, w2h = hc[0], hc[1]
            for s in range(8):
                rloc = t * 8 + s
                nc.tensor.matmul(
                    ps2[:, rloc * 8:rloc * 8 + 8],
                    rr[:, s * 128:(s + 1) * 128], w2h)
            hc[2] += 1
            last = hc[3] == BL - 1 and hc[4] == 1
            if last and hc[2] == 8:
                # final half: split exp so the AV chain starts earlier
                wsb = wsb_p.tile([128, 1024], FP16, tag="wsb")
                hc.append(wsb)
                nc.scalar.activation(wsb[:, 0:512], ps2[:, 0:512], AF.Exp)
                bal.charge("A", 512)
            if hc[2] == 16:   # half fully reduced -> exp + schedule tail
                if last:
                    wsb = hc[5]
                    nc.scalar.activation(wsb[:, 512:1024], ps2[:, 512:1024],
                                         AF.Exp)
                    bal.charge("A", 512)
                    return make_tail(hc[3], hc[4], wsb, split=True)
                wsb = wsb_p.tile([128, 1024], FP16, tag="wsb")
                nc.scalar.activation(wsb[:], ps2[:], AF.Exp)
                bal.charge("A", 1024)
                return make_tail(hc[3], hc[4], wsb)
            return None

        for b in range(BL):
            if b + 2 < BL:
                rhss[b + 2] = assemble(b + 2)
            rhs = rhss.pop(b)
            rhs3 = rhs[:].rearrange("p (g n) -> p g n", g=2)
            for half in range(2):
                w1h = w1v[:, half]            # [9, 2, 128]
                w2h = w2sb[:, half * 8:(half + 1) * 8]
                bch = bc2[:, half:half + 1]
                halfctx = [None, w2h, 0, b, half]
                for t in range(16):   # 16 tiles of 1024 points per half
                    ps1 = psA.tile([128, 1024], FP32, tag="psA")
                    for k4 in range(2):
                        n0 = t * 1024 + k4 * 512
                        nc.tensor.matmul(
                            ps1[:, k4 * 512:(k4 + 1) * 512], w1h,
                            rhs3[:, :, n0:n0 + 512], perf_mode=DR)
                    rr = rr_p.tile([128, 1024], FP16, tag="rr")
                    bal.relu(rr[:], ps1[:], bch, 1024)
                    pend.append((rr, t, halfctx))
                    if len(pend) > 4:
                        nt = emit_l2(*pend.pop(0))
                        if nt is not None:
                            prev_tail = nt
                    if t == 12 and prev_tail is not None:
                        prev_tail()
                        prev_tail = None

        while pend:
            nt = emit_l2(*pend.pop(0))
            if nt is not None:
                prev_tail = nt
        prev_tail()

    nc.compile()
    return nc


_cache = {}


def _prep(inputs):
    import ml_dtypes
    FP8NP = ml_dtypes.float8_e4m3fn

    row_emb = np.asarray(inputs["row_emb"], dtype=np.float32)
    col_emb = np.asarray(inputs["col_emb"], dtype=np.float32)
    cost_mat = np.asarray(inputs["cost_mat"], dtype=np.float32)
    Wq = np.asarray(inputs["Wq"], dtype=np.float32) / np.sqrt(D)
    Wk = np.asarray(inputs["Wk"], dtype=np.float32)
    Wv = np.asarray(inputs["Wv"], dtype=np.float32)
    m1w = np.asarray(inputs["mix1_weight"], dtype=np.float32)
    m1b = np.asarray(inputs["mix1_bias"], dtype=np.float32)
    m2w = np.asarray(inputs["mix2_weight"], dtype=np.float32)

    a1, c1 = m1w[:, 0, :], m1w[:, 1, :]
    w2 = m2w[:, :, 0]

    # layer1 stationary fp8, DoubleRow grouped: w1l8[p, (half, g, col)]
    # g0: heads 0..7 at p0..7, cost at p8 (c1 weights)
    # g1: heads 8..15 at p0..7, cost-dup at p8 (zero weights)
    w1l8 = np.zeros((9, 2, 2, 128), dtype=np.float32)
    w2l = np.zeros((128, 16), dtype=np.float16)
    bcol2 = np.zeros((128, 2), dtype=np.float32)
    for h in range(H):
        half, hl = h // 8, h % 8
        g, p = h // 8, h % 8
        for m in range(MS):
            col = hl * 16 + m
            w1l8[p, half, g, col] = a1[h, m]
            w1l8[8, half, 0, col] = c1[h, m]     # cost row -> (g0, p8)
            w2l[hl * 16 + m, half * 8 + hl] = w2[h, m]
            bcol2[hl * 16 + m, half] = m1b[h, m]
    w1l8 = w1l8.reshape(9, 512).astype(FP8NP)

    # q/k weights: head h -> tile h%4 (128-col block), slot h//4 (32 cols)
    def pad_qk(w):
        wp = np.zeros((E, 512), dtype=np.float16)
        w4 = w.reshape(E, H, D)
        for h in range(H):
            base = (h % 4) * 128 + (h // 4) * 32
            wp[:, base:base + 16] = w4[:, h, :]
        return wp

    def fold_eh(a):
        # [E, X] -> [128, (eh, X)]: row eh*128+p -> partition p, col-block eh
        x = a.reshape(2, 128, a.shape[1]).transpose(1, 0, 2)
        return np.ascontiguousarray(x.reshape(128, -1).astype(np.float16))

    wqk = fold_eh(np.concatenate([pad_qk(Wq), pad_qk(Wk)], axis=1))
    wv = fold_eh(Wv)

    per_core = []
    for i in range(NCORES):
        sl = slice(i * BL, (i + 1) * BL)
        xr = row_emb[sl].reshape(TOK, E).T
        xc = col_emb[sl].reshape(TOK, E).T
        per_core.append({
            "xrT": fold_eh(xr),
            "xcT": fold_eh(xc),
            "wqk": wqk,
            "wv": wv,
            "w1l8": w1l8,
            "w2l": w2l,
            "bcol2": bcol2,
            "cost8": cost_mat[sl].reshape(BL, PTS).astype(FP8NP),
        })
    return per_core


def kernel(**inputs):
    if "nc" not in _cache:
        _cache["nc"] = build_kernel()
    nc = _cache["nc"]
    in_maps = _prep(inputs)
    res = run_bass_kernel_spmd(nc, in_maps, list(range(NCORES)))
    out = np.concatenate([res.results[i]["out"] for i in range(NCORES)],
                         axis=0)
    return out.astype(np.float32)
